# revision 39
# baseline (speedup 1.0000x reference)
"""Trainium2 Bass kernel for the nn_Dynamics problem.

Math (per batch element, d=8, H=128):
  x = X[:, :8], v = X[:, 8:]
  z0 = W0 x + b0; h0 = tanh(z0); z1 = W1 h0 + b1; h1 = tanh(z1)
  a1 = (1-h1^2)*w2;  A0 = W1^T a1;  a0 = (1-h0^2)*A0;  g = W0^T a0
  t0 = W0 v; t1 = W1((1-h0^2) t0)
  hvv = -2 sum_k [a1 h1 t1^2 + A0 h0 (1-h0^2) t0^2]
  force = -(K x + D v)
  out = force - g*(g.force + hvv)/(1 + |g|^2)      (Sherman-Morrison)

Device mapping:
  - Host pre-transposes X (f16), with a batch permutation (col 128J+p holds
    X row 64p+J) so the final out DMA is 2KB-contiguous per partition.
  - w2 and the -1 of d1=(1-h1^2) are folded into the A0 stationary:
    A0raw = (W1*w2)^T h1sq, A0n = A0raw - c0 = -W1^T a1;  c0 column is
    subtracted inside the custom consumer ops.
  - e1n = -e1 (w2 negated), ecomb = e1n - e2 (GpSimd), hv = sum_k(ecomb)
    via GpSimd partition_all_reduce (= hvv/2, sign folded into M).
  - g via one m=8 PE matmul; per-tile g rows + hv row staged next to X^T
    rows in E_all (f16); per 4-tile group, 16 tiny transpose-matmuls with
    the augmented moving map M compute [force, g, hv] batch-major in one
    psum bank; one scalar copy stages it for the batch-major tail.
Sharding: pure data parallel over 8 NeuronCores (8192 rows each).
"""

import os

import numpy as np

import concourse.bacc as bacc
import concourse.bass as bass
import concourse.bass_isa as bass_isa
import concourse.dve_ops as dve_ops
import concourse.tile as tile
from concourse import mybir
from concourse.bass_utils import run_bass_kernel_spmd
from concourse.dve_ops import DveOp
from concourse.dve_ops import has_src1
from concourse.dve_spec import C0, C2, One, Spec, Src0, Src1, lower, sq
from concourse.dve_uop import DveOpSpec

F32 = mybir.dt.float32
F16 = mybir.dt.float16
AX = mybir.AxisListType
OP = mybir.AluOpType
ACT = mybir.ActivationFunctionType

DIM = 8
H = 128
BATCH = 65536
NCORES = 8
BC = BATCH // NCORES          # 8192 rows per core
TW = 512                      # batch tile width
NT = BC // TW                 # 16 tiles per core
NCH = TW // 128               # 4 chunks of 128 per tile
NBT = 4                       # tiles per tail group
NG = NT // NBT                # tail groups
CW = 2 * DIM + 1              # 17 packed batch-major cols per chunk
GW = NBT * NCH * CW           # 272 bm cols per group

LAST_RESULTS = None

# ---------------- custom fused DVE ops ----------------


def _register_op(name, body, reference):
    if name in dve_ops._SUB_OPCODE_FOR_NAME:
        for op in dve_ops.OPS:
            if op.name == name:
                return op
    spec = Spec(body=body, reference=reference)
    shas = {}
    for ver in ("v3", "v4"):
        shas[ver] = DveOpSpec(
            name=name,
            opcode=dve_ops._CUSTOM_DVE_ROW_BASE + len(dve_ops.OPS),
            uops=lower(spec, ver=ver),
            rd1_en=has_src1(spec),
        ).sha(ver)
    op = DveOp(name, spec, subdim=False, uops_sha=shas)
    dve_ops.OPS.append(op)
    dve_ops.CUSTOM_DVE_SPECS[name] = spec
    dve_ops._SUB_OPCODE_FOR_NAME[name] = (
        dve_ops._CUSTOM_DVE_ROW_BASE + len(dve_ops.OPS) - 1
    )
    return op


# h0p' = (h0^2 - 1) * t0
OP_SQM1_MUL = _register_op(
    "ANT_SQM1_MUL",
    (sq(Src0) - One) * Src1,
    lambda in0, in1: (in0 * in0 - 1.0) * in1,
)
# u' = h0 * (h0^2 - 1) * t0^2
OP_UPRIME = _register_op(
    "ANT_UPRIME",
    Src0 * (sq(Src0) - One) * sq(Src1),
    lambda in0, in1: in0 * (in0 * in0 - 1.0) * in1 * in1,
)
# e1 = (1 - h1^2) * w2 * h1 * t1^2
OP_E1F = _register_op(
    "ANT_E1F",
    (One - sq(Src0)) * C0 * Src0 * sq(Src1),
    lambda in0, in1, s0: (1.0 - in0 * in0) * s0 * in0 * in1 * in1,
)
# gsq with 1/8 folded in: sum over 8 features gives 1+|g|^2 directly
OP_SQP = _register_op(
    "ANT_SQP",
    sq(Src0) + C2,
    lambda in0, imm2: in0 * in0 + imm2,
)
# a0 = (h0^2-1) * (A0raw - c0)
OP_SQM1_MULS = _register_op(
    "ANT_SQM1_MULS",
    (sq(Src0) - One) * (Src1 - C0),
    lambda in0, in1, s0: (in0 * in0 - 1.0) * (in1 - s0),
)
# e2 = (A0raw - c0) * u
OP_MULS = _register_op(
    "ANT_MULS",
    (Src0 - C0) * Src1,
    lambda in0, in1, s0: (in0 - s0) * in1,
)

# f16 weight blob layout (free-axis offsets)
B_W0TX = 0          # [16, 128]
B_W0TV = 128        # [16, 128]
B_M = 256           # [32, 17]
B_W1T = 288         # [128, 128]
B_W1W = 416         # [128, 128]
B_W016 = 544        # [128, 16] (W0 | zeros)
B_P216 = 560        # [128, 16] (zeros | 2.0)
B_F16 = 576


def build_nc():
    nc = bacc.Bacc()

    XT16 = nc.dram_tensor("XT16", [2 * DIM, BC], F16, kind="ExternalInput")
    WB16 = nc.dram_tensor("WB16", [128, B_F16], F16, kind="ExternalInput")
    WB32 = nc.dram_tensor("WB32", [128, 4], F32, kind="ExternalInput")
    out = nc.dram_tensor("out", [BC, DIM], F32, kind="ExternalOutput")

    from contextlib import ExitStack

    with tile.TileContext(nc) as tc, ExitStack() as stk:
        consts = stk.enter_context(tc.tile_pool(name="consts", bufs=1))
        work = stk.enter_context(tc.tile_pool(name="work", bufs=3))
        tbp = stk.enter_context(tc.tile_pool(name="tbp", bufs=2))
        ps_zz = stk.enter_context(tc.tile_pool(name="ps_zz", bufs=2, space="PSUM"))
        ps_tt = stk.enter_context(tc.tile_pool(name="ps_tt", bufs=2, space="PSUM"))
        ps_aa = stk.enter_context(tc.tile_pool(name="ps_aa", bufs=2, space="PSUM"))
        ps_fm = stk.enter_context(tc.tile_pool(name="ps_fm", bufs=1, space="PSUM"))
        ps_bm = stk.enter_context(tc.tile_pool(name="ps_bm", bufs=1, space="PSUM"))

        # ---------------- constants (weights first: first z0 needs them) ----
        wb16 = consts.tile([128, B_F16], F16)
        nc.sync.dma_start(out=wb16, in_=WB16[:, :])
        wb32 = consts.tile([128, 4], F32)
        nc.sync.dma_start(out=wb32, in_=WB32[:, :])

        XT_sb = consts.tile([2 * DIM, BC], F16)
        nc.sync.dma_start(out=XT_sb[:, 0 : 2 * TW], in_=XT16[:, 0 : 2 * TW])
        nc.sync.dma_start(out=XT_sb[:, 2 * TW :], in_=XT16[:, 2 * TW :])
        # E_all rows: 0:8 = g (per tile), 8 = hv, 16:32 = X^T (f16)
        E_all = consts.tile([32, BC], F16)
        nc.sync.dma_start(out=E_all[16:32, :], in_=XT16[:, :])

        W0Tx_sb = wb16[0:16, B_W0TX : B_W0TX + 128]
        W0Tv_sb = wb16[0:16, B_W0TV : B_W0TV + 128]
        M_sb = wb16[0:32, B_M : B_M + CW]
        W1T_sb = wb16[:, B_W1T : B_W1T + 128]
        W1w_sb = wb16[:, B_W1W : B_W1W + 128]
        W016_sb = wb16[:, B_W016 : B_W016 + 16]
        P216_sb = wb16[:, B_P216 : B_P216 + 16]
        b0_sb = wb32[:, 0:1]
        b1_sb = wb32[:, 1:2]
        c0_sb = wb32[:, 2:3]
        w2n_sb = wb32[:, 3:4]

        out_sb = consts.tile([128, (BC // 128) * DIM], F32)

        # preload the activation table while input DMAs are in flight
        warm = consts.tile([128, 1], F16)
        nc.vector.memset(warm, 0.0)
        warm2 = consts.tile([128, 1], F16)
        nc.scalar.activation(warm2, warm, ACT.Tanh)

        # ---------------- pipelined main loop ----------------
        state = {}

        def front(t):
            sl = slice(TW * t, TW * (t + 1))

            z0 = ps_zz.tile([H, TW], F32, tag="zz")
            nc.tensor.matmul(z0, W0Tx_sb, XT_sb[:, sl], start=True, stop=True)
            t0 = ps_tt.tile([H, TW], F32, tag="tt")
            nc.tensor.matmul(t0, W0Tv_sb, XT_sb[:, sl], start=True, stop=True)

            h0 = work.tile([H, TW], F16, tag="h0")
            nc.scalar.activation(h0, z0, ACT.Tanh, bias=b0_sb, scale=1.0)

            h0p = work.tile([H, TW], F16, tag="h0p")
            nc.vector._custom_dve(OP_SQM1_MUL, out=h0p, in0=h0, in1=t0[:, :])
            u = work.tile([H, TW], F16, tag="u")
            nc.vector._custom_dve(OP_UPRIME, out=u, in0=h0, in1=t0[:, :])

            z1 = ps_zz.tile([H, TW], F32, tag="zz")
            nc.tensor.matmul(z1, W1T_sb, h0, start=True, stop=True)
            t1 = ps_tt.tile([H, TW], F32, tag="tt")
            nc.tensor.matmul(t1, W1T_sb, h0p, start=True, stop=True)

            h1 = work.tile([H, TW], F16, tag="h1")
            nc.scalar.activation(h1, z1, ACT.Tanh, bias=b1_sb, scale=1.0)

            # h1sq on the Activation engine (same act table as tanh)
            h1sq = work.tile([H, TW], F16, tag="h1sq")
            nc.scalar.activation(h1sq, h1, ACT.Square)

            # e1n = -e1  (w2 negated via s0)
            e1 = work.tile([H, TW], F16, tag="e1")
            nc.vector._custom_dve(
                OP_E1F, out=e1, in0=h1, in1=t1[:, :], s0=w2n_sb
            )

            A0 = ps_aa.tile([H, TW], F32, tag="aa")
            nc.tensor.matmul(A0, W1w_sb, h1sq, start=True, stop=True)

            # a0 = (h0^2-1)*(A0raw-c0) = +(1-h0^2) W1^T a1 (true sign)
            a0 = work.tile([H, TW], F16, tag="a0")
            nc.vector._custom_dve(
                OP_SQM1_MULS, out=a0, in0=h0, in1=A0[:, :], s0=c0_sb
            )
            e2 = work.tile([H, TW], F16, tag="e2")
            nc.vector._custom_dve(
                OP_MULS, out=e2, in0=A0[:, :], in1=u, s0=c0_sb
            )
            # ecomb = e1n - e2 = -e1 - e2 on GpSimd (plain TT only)
            ecomb = work.tile([H, TW], F16, tag="ec")
            nc.gpsimd.tensor_sub(ecomb, e1, e2)

            # feature-major mini-block: g rows 0:8 first (a0 ready early),
            # hv rows 8:16 accumulated on top (disjoint stationary columns)
            fm = ps_fm.tile([16, TW], F32, tag="fm")
            nc.tensor.matmul(fm, W016_sb, a0, start=True, stop=False)
            nc.tensor.matmul(fm, P216_sb, ecomb, start=False, stop=True)
            state[t] = fm

        def stage2(t):
            sl = slice(TW * t, TW * (t + 1))
            fm = state.pop(t)
            # stage g/hv rows next to X^T rows for the fused transpose
            nc.scalar.copy(E_all[0:16, sl], fm)

        def groupstage(g):
            # fused transpose + force map: one psum bank per group
            bm = ps_bm.tile([128, GW], F32, tag="bm")
            for j in range(NBT * NCH):
                nc.tensor.matmul(
                    bm[:, CW * j : CW * (j + 1)],
                    E_all[:, NBT * TW * g + 128 * j : NBT * TW * g + 128 * (j + 1)],
                    M_sb,
                    start=True,
                    stop=True,
                )
            tb4 = tbp.tile([128, GW], F32, tag="tb", name="tb4")
            nc.scalar.copy(tb4, bm)

            # ---- batched batch-major tail ----
            B = NBT * NCH  # 16 chunks
            def col3(off, w):
                return bass.AP(
                    tensor=tb4.tensor,
                    offset=tb4.offset + off,
                    ap=[list(tb4.ap[0]), [CW, B], [1, w]],
                )

            f3 = col3(0, DIM)
            gn3 = col3(DIM, DIM)
            hv2 = bass.AP(
                tensor=tb4.tensor,
                offset=tb4.offset + 2 * DIM,
                ap=[list(tb4.ap[0]), [CW, B]],
            )

            gb = tbp.tile([128, 2 * B * DIM], F32, tag="gb")
            gb4 = gb.rearrange("p (q j f) -> p q j f", q=2, f=DIM)
            nc.vector._custom_dve(OP_SQP, out=gb4[:, 0], in0=gn3, imm2=1.0 / DIM)
            nc.gpsimd.tensor_mul(gb4[:, 1], gn3, f3)
            red = tbp.tile([128, 2 * B], F32, tag="red")
            red3 = red.rearrange("p (q j) -> p q j", q=2)
            nc.vector.tensor_reduce(red3, gb4, axis=AX.X, op=OP.add)
            num = tbp.tile([128, B], F32, tag="num")
            nc.vector.tensor_sub(num, hv2, red3[:, 1])
            rec = tbp.tile([128, B], F32, tag="rec")
            nc.vector.reciprocal(rec, red3[:, 0])
            s = tbp.tile([128, B], F32, tag="s")
            nc.vector.tensor_mul(s, num, rec)
            sbc = bass.AP(
                tensor=s.tensor,
                offset=s.offset,
                ap=[list(s.ap[0]), [1, B], [0, DIM]],
            )
            su = tbp.tile([128, B * DIM], F32, tag="su")
            su3 = su.rearrange("p (j f) -> p j f", f=DIM)
            nc.gpsimd.tensor_mul(su3, gn3, sbc)
            ob = out_sb[:, B * DIM * g : B * DIM * (g + 1)]
            nc.gpsimd.tensor_add(
                ob.rearrange("p (j f) -> p j f", f=DIM), f3, su3
            )
            # stream this group's output slice out immediately
            nc.sync.dma_start(
                out=out.rearrange("(p j) f -> p (j f)", p=128)[
                    :, B * DIM * g : B * DIM * (g + 1)
                ],
                in_=ob,
            )

        for t in range(NT):
            front(t)
            if t >= 1:
                stage2(t - 1)
            if t % NBT == 0 and t >= NBT:
                groupstage(t // NBT - 1)
        stage2(NT - 1)
        groupstage(NG - 1)

    if not nc.is_finalized():
        nc.finalize()

    return nc


_NC_CACHE = None


def _install_ntff_shim():
    """Register the axon NTFF profile hook (missing antenv.axon_hooks shim)."""
    import sys
    import types

    if "antenv.axon_hooks" in sys.modules:
        return
    try:
        sys.path.insert(0, "/root/.axon_site")
        from trn_agent_boot.trn_boot import _ntff_profile_via_ctypes

        hook = _ntff_profile_via_ctypes("/opt/axon/libaxon_pjrt.so")
        mod = types.ModuleType("antenv.axon_hooks")
        mod.get_axon_ntff_profile_hook = lambda: hook
        sys.modules["antenv.axon_hooks"] = mod
    except Exception:
        pass


def kernel(**inputs):
    global LAST_RESULTS, _NC_CACHE
    trace = bool(int(os.environ.get("KERNEL_TRACE", "0")))
    if trace:
        _install_ntff_shim()
    if _NC_CACHE is None:
        _NC_CACHE = build_nc()
    nc = _NC_CACHE

    X = np.ascontiguousarray(inputs["X"], dtype=np.float32)
    K = np.asarray(inputs["K"], np.float32)
    D = np.asarray(inputs["D"], np.float32)
    W0 = np.asarray(inputs["W0"], np.float32)
    W1 = np.asarray(inputs["W1"], np.float32)
    W2 = np.asarray(inputs["W2"], np.float32)

    w1w16 = (W1 * W2.reshape(H, 1)).astype(np.float16)
    c0 = w1w16.astype(np.float32).sum(axis=0).reshape(H, 1)

    wb16 = np.zeros((128, B_F16), np.float32)
    wb16[0:DIM, B_W0TX : B_W0TX + 128] = W0.T
    wb16[DIM : 2 * DIM, B_W0TV : B_W0TV + 128] = W0.T
    # M: col q<8: force map (-K^T on x rows 16:24, -D^T on v rows 24:32);
    # cols 8:16 pass g rows 0:8; col 16 = -2*hv row 8 (hv = hvv/2, negated).
    wb16[16:24, B_M : B_M + DIM] = -K.T
    wb16[24:32, B_M : B_M + DIM] = -D.T
    for i in range(DIM):
        wb16[i, B_M + DIM + i] = 1.0
    wb16[DIM, B_M + 2 * DIM] = -1.0  # hv_row = 2*sum(ecomb) = hvv; negate
    wb16[:, B_W1T : B_W1T + 128] = W1.T
    wb16[:, B_W1W : B_W1W + 128] = w1w16.astype(np.float32)
    wb16[:, B_W016 : B_W016 + DIM] = W0
    wb16[:, B_P216 + DIM : B_P216 + 16] = 2.0

    wb32 = np.zeros((128, 4), np.float32)
    wb32[:, 0] = np.asarray(inputs["b0"], np.float32)
    wb32[:, 1] = np.asarray(inputs["b1"], np.float32)
    wb32[:, 2] = c0[:, 0]
    wb32[:, 3] = -W2.reshape(H)

    shared = {"WB16": wb16.astype(np.float16), "WB32": wb32}

    # batch permutation: device column 128J+p holds X row 64p+J so the
    # output DMA is 2KB-contiguous per partition.
    b = np.arange(BC)
    perm = (BC // 128) * (b % 128) + b // 128

    in_maps = []
    for i in range(NCORES):
        xp = X[i * BC : (i + 1) * BC][perm]
        m = {"XT16": np.ascontiguousarray(xp.T).astype(np.float16)}
        m.update(shared)
        in_maps.append(m)

    res = run_bass_kernel_spmd(
        nc, in_maps, core_ids=list(range(NCORES)), trace=trace
    )
    LAST_RESULTS = res
    out_full = np.concatenate([res.results[i]["out"] for i in range(NCORES)], axis=0)
    return out_full.astype(np.float32)
